# revision 4
# baseline (speedup 1.0000x reference)
# Trainium2 Bass kernel for nn_DatasetProjector (moe_routing).
#
# Math: out[b] = gelu( conv1d(pad(x[b]), W[d_b], k=3, pad=1) + bias[d_b] )
# where d_b = dataset_id[b] selects one of 8 "dataset experts" (512 out
# channels each) from the 4096-channel conv weight.  Only the selected
# expert's 512 channels are computed (8x less work than the full conv).
#
# Sharding: data-parallel over batch, 4 examples per core across 8 cores.
# The per-example expert weight slice is gathered on host (routing), laid
# out as lhsT [c, k, m] so the conv is 6 accumulating matmuls per PSUM
# tile (2 channel-tiles x 3 taps; the tap shift is a free SBUF column
# offset into the zero-padded x tile).

import os
from contextlib import ExitStack

import numpy as np

import concourse.bass as bass
import concourse.tile as tile
from concourse import bacc, mybir
from concourse.bass_utils import run_bass_kernel_spmd

B = 32          # batch
CIN = 200       # input channels (pad-to-256 channels multiply zeros; dropped)
T = 1024        # time
M = 512         # model dim (selected expert's out channels)
NUM_DATASETS = 8
NCORES = 8
NB = B // NCORES   # examples per core
TPAD = T + 2       # one zero column each side for the k=3 taps
C0 = 128           # first channel tile
C1 = CIN - C0      # second channel tile (72)
NMT = M // 128     # output-channel tiles (4)
NTT = T // 512     # time tiles (2)

# Matmul operand dtype: float32 is the safe baseline (4 cyc/row);
# float32r uses fp32-replication transpose mode (1 cyc/row at N>=512).
MM_DT = getattr(mybir.dt, os.environ.get("KERNEL_MM_DT", "float32r"))
ACC_DT = mybir.dt.float32


def _build_nc():
    nc = bacc.Bacc("TRN2", target_bir_lowering=False, debug=False)
    x_d = nc.dram_tensor("x", [NB, CIN, TPAD], MM_DT, kind="ExternalInput").ap()
    wa_d = nc.dram_tensor("wa", [NB, C0, 3, M], MM_DT, kind="ExternalInput").ap()
    wb_d = nc.dram_tensor("wb", [NB, C1, 3, M], MM_DT, kind="ExternalInput").ap()
    bias_d = nc.dram_tensor("bias", [128, NB * NMT], ACC_DT, kind="ExternalInput").ap()
    out_d = nc.dram_tensor("out", [NB, M, T], ACC_DT, kind="ExternalOutput").ap()

    with tile.TileContext(nc) as tc, ExitStack() as ctx:
        xpool = ctx.enter_context(tc.tile_pool(name="xpool", bufs=2))
        wpool = ctx.enter_context(tc.tile_pool(name="wpool", bufs=2))
        bpool = ctx.enter_context(tc.tile_pool(name="bpool", bufs=1))
        pspool = ctx.enter_context(tc.tile_pool(name="pspool", bufs=8, space="PSUM"))
        opool = ctx.enter_context(tc.tile_pool(name="opool", bufs=4))

        b_sb = bpool.tile([128, NB * NMT], ACC_DT)
        nc.sync.dma_start(out=b_sb, in_=bias_d)

        for i in range(NB):
            xa = xpool.tile([C0, TPAD], MM_DT, tag="xa")
            nc.sync.dma_start(out=xa, in_=x_d[i, 0:C0, :])
            xb = xpool.tile([C1, TPAD], MM_DT, tag="xb")
            nc.sync.dma_start(out=xb, in_=x_d[i, C0:CIN, :])
            wa = wpool.tile([C0, 3, M], MM_DT, tag="wa")
            nc.sync.dma_start(out=wa, in_=wa_d[i])
            wb = wpool.tile([C1, 3, M], MM_DT, tag="wb")
            nc.sync.dma_start(out=wb, in_=wb_d[i])

            for mt in range(NMT):
                for tt in range(NTT):
                    ps = pspool.tile([128, 512], ACC_DT, tag="ps")
                    for k in range(3):
                        nc.tensor.matmul(
                            ps,
                            lhsT=wa[:, k, mt * 128:(mt + 1) * 128],
                            rhs=xa[:, k + tt * 512: k + tt * 512 + 512],
                            start=(k == 0),
                            stop=False,
                        )
                        nc.tensor.matmul(
                            ps,
                            lhsT=wb[:, k, mt * 128:(mt + 1) * 128],
                            rhs=xb[:, k + tt * 512: k + tt * 512 + 512],
                            start=False,
                            stop=(k == 2),
                        )
                    o = opool.tile([128, 512], ACC_DT, tag="o")
                    nc.scalar.activation(
                        o, ps, mybir.ActivationFunctionType.Gelu,
                        bias=b_sb[:, i * NMT + mt: i * NMT + mt + 1],
                    )
                    nc.sync.dma_start(
                        out=out_d[i, mt * 128:(mt + 1) * 128,
                                  tt * 512:(tt + 1) * 512],
                        in_=o,
                    )
    nc.compile()
    return nc


_NC = None


def _get_nc():
    global _NC
    if _NC is None:
        _NC = _build_nc()
    return _NC


def _prepare_in_maps(x, dataset_id, W, b):
    x = np.ascontiguousarray(np.asarray(x, dtype=np.float32))
    d = np.asarray(dataset_id).astype(np.int64)
    W = np.asarray(W, dtype=np.float32)
    b = np.asarray(b, dtype=np.float32)

    # lhsT layout per expert: [c, k, m] = W[e*M + m, c, k]
    wt = np.ascontiguousarray(
        W.reshape(NUM_DATASETS, M, W.shape[1], 3)[:, :, :CIN, :]
        .transpose(0, 2, 3, 1)  # (8, CIN, 3, M)
    )
    wt_sel = wt[d]  # (B, CIN, 3, M)

    xp = np.zeros((B, CIN, TPAD), dtype=np.float32)
    xp[:, :, 1:T + 1] = x

    # bias tile layout: [p, i*NMT + mt] = b[d_i*M + mt*128 + p]
    b_sel = b.reshape(NUM_DATASETS, M)[d]          # (B, M)
    b_tiles = b_sel.reshape(B, NMT, 128).transpose(2, 0, 1)  # (128, B, NMT)

    in_maps = []
    for c in range(NCORES):
        sl = slice(c * NB, (c + 1) * NB)
        in_maps.append({
            "x": xp[sl],
            "wa": np.ascontiguousarray(wt_sel[sl, :C0]),
            "wb": np.ascontiguousarray(wt_sel[sl, C0:]),
            "bias": np.ascontiguousarray(
                b_tiles[:, sl, :].reshape(128, NB * NMT)),
        })
    return in_maps


def run(x, dataset_id, W, b, trace=False):
    """Returns (out, BassKernelResults)."""
    nc = _get_nc()
    in_maps = _prepare_in_maps(x, dataset_id, W, b)
    res = run_bass_kernel_spmd(nc, in_maps, list(range(NCORES)), trace=trace)
    out = np.concatenate([res.results[c]["out"] for c in range(NCORES)], axis=0)
    return out, res


def kernel(x, dataset_id, W, b):
    out, _ = run(x, dataset_id, W, b)
    return out
